# revision 2
# baseline (speedup 1.0000x reference)
"""Trainium2 Bass kernel: split-precision x stream with exact weights.

x is decomposed on host as x = hi + lo*2^-S with hi = fp16(x) and the
residual lo sent either as fp16 (4 B/elem total, ~fp32-exact) or fp8e4m3
(3 B/elem, ~15-bit x fidelity).  Weights keep full precision via a packed
stationary [W16 | (W - W16)*2^8]: the Wres columns ride the same matmuls
for free and are folded in with a scaled add.  Per timestep-pair:
  tile1 [20, 512] <- bias MM + 8 hi-strip MMs  (rows 0:10 W16, 10:20 Wres)
  tile2 [10, 512] <- 8 lo-strip MMs
  sn = t1a + 2^-8*(t1b + t2*2^-(S_lo-8... folded scalars))
LIF scan: fold on GPSIMD+DVE, mem chain on DVE, spike Sign on ACT,
time-fuse accumulate on GPSIMD.  2-class softmax as sigmoid of the
logit difference.
"""

import os
import sys
import types
from contextlib import ExitStack

import numpy as np

for _p in ("/opt/trn_rl_repo", "/root/.axon_site/_ro/trn_rl_repo"):
    if _p not in sys.path and os.path.isdir(_p):
        sys.path.insert(0, _p)


def _install_ntff_shim():
    try:
        import antenv.axon_hooks  # noqa: F401
        return
    except ImportError:
        pass
    try:
        import antenv
        from trn_agent_boot.trn_boot import _ntff_profile_via_ctypes
        hook = _ntff_profile_via_ctypes("/opt/axon/libaxon_pjrt.so")
        mod = types.ModuleType("antenv.axon_hooks")
        mod.get_axon_ntff_profile_hook = lambda: hook
        mod.set_axon_ntff_profile_hook = lambda h: None
        sys.modules["antenv.axon_hooks"] = mod
        antenv.axon_hooks = mod
    except Exception:
        pass


_install_ntff_shim()

import concourse.bacc as bacc
import concourse.bass as bass
import concourse.mybir as mybir
import concourse.tile as tile
from concourse.bass_utils import run_bass_kernel_spmd

F32 = mybir.dt.float32
F16 = mybir.dt.float16
FP8 = mybir.dt.float8e4
ALU = mybir.AluOpType
ACTF = mybir.ActivationFunctionType

B, T, A, D, H, O = 2048, 90, 4, 256, 10, 2
N_CORES = 8
BS = B // N_CORES          # 256 batch rows per core
AD = A * D                 # 1024
NSTRIP = AD // 128         # 8
BETA = 0.95
THR = 1.0
NT = 6                     # timesteps per DMA group
NGROUP = T // NT

LO_FP8 = True              # False: lo fp16 (4B/elem); True: lo fp8 (3B/elem)
S_LO = 9 if LO_FP8 else 10         # lo = (x - hi) * 2^S_LO
S_FOLD = 16                        # both correction channels fold by 2^-16
S_WLO = S_FOLD - S_LO              # lo-stream weights scaled by 2^S_WLO
LO_DT = FP8 if LO_FP8 else F16


def _build(scalars):
    w_time = scalars["w_time"]

    nc = bacc.Bacc()
    xhi_d = nc.dram_tensor("xhi", (128, T, 2 * AD), F16, kind="ExternalInput")
    xlo_d = nc.dram_tensor("xlo", (128, T, 2 * AD), LO_DT, kind="ExternalInput")
    whi_d = nc.dram_tensor("whi", (128, 336), F16, kind="ExternalInput")
    wlo_d = nc.dram_tensor("wlo", (128, 336), LO_DT, kind="ExternalInput")
    wext_d = nc.dram_tensor("wext", (1, 554), F16, kind="ExternalInput")
    wf_d = nc.dram_tensor("wf32", (10, 4), F32, kind="ExternalInput")
    out_d = nc.dram_tensor("out", (O, BS), F32, kind="ExternalOutput")

    with ExitStack() as ctx:
        tc = ctx.enter_context(tile.TileContext(nc))
        consts = ctx.enter_context(tc.tile_pool(name="consts", bufs=1))
        xph = ctx.enter_context(tc.tile_pool(name="xph", bufs=3))
        xpl = ctx.enter_context(tc.tile_pool(name="xpl", bufs=3))
        state = ctx.enter_context(tc.tile_pool(name="state", bufs=2))
        outp = ctx.enter_context(tc.tile_pool(name="outp", bufs=1))
        ps1 = ctx.enter_context(tc.tile_pool(name="ps1", bufs=6, space="PSUM"))
        ps_lg = ctx.enter_context(tc.tile_pool(name="ps_lg", bufs=1, space="PSUM"))

        whi = consts.tile([128, 336], F16)
        nc.sync.dma_start(out=whi, in_=whi_d[:, :])
        wlo = consts.tile([128, 336], LO_DT)
        nc.sync.dma_start(out=wlo, in_=wlo_d[:, :])
        wext = consts.tile([1, 554], F16)
        nc.sync.dma_start(out=wext, in_=wext_d[:, :])
        wf32 = consts.tile([10, 4], F32)
        nc.sync.dma_start(out=wf32, in_=wf_d[:, :])
        ones512 = wext[0:1, 0:512]
        bias42 = wext[0:1, 512:554]               # [bc16 | 0*22 | bcres']
        wdiff = wf32[0:10, 0:1]
        csig_p = wf32[0:1, 1:2]
        csig_n = wf32[0:1, 2:3]
        negthr = wf32[0:10, 3:4]

        mem = state.tile([10, BS], F32, tag="mem")
        nc.vector.memset(mem, 0.0)
        ft = state.tile([10, BS], F32, tag="ft")
        nc.vector.memset(ft, 0.0)

        for g in range(NGROUP):
            t0 = g * NT
            xh = xph.tile([128, NT, 2 * AD], F16, tag="xh")
            nc.sync.dma_start(out=xh, in_=xhi_d[:, t0:t0 + NT, :])
            xl = xpl.tile([128, NT, 2 * AD], LO_DT, tag="xl")
            nc.scalar.dma_start(out=xl, in_=xlo_d[:, t0:t0 + NT, :])
            for sp in range(NT // 2):
                s0 = 2 * sp
                t = t0 + s0
                t1 = ps1.tile([42, 2 * BS], F32, tag="t1")
                nc.tensor.matmul(
                    t1, lhsT=bias42, rhs=ones512,
                    start=True, stop=False, skip_group_check=True)
                for c in range(NSTRIP):
                    nc.tensor.matmul(
                        t1,
                        lhsT=whi[:, c * 42:(c + 1) * 42],
                        rhs=xh[:, s0:s0 + 2, c * BS:(c + 1) * BS],
                        start=False, stop=False,
                        skip_group_check=True)
                for c in range(NSTRIP):
                    nc.tensor.matmul(
                        t1,
                        lhsT=wlo[:, c * 42:(c + 1) * 42],
                        rhs=xl[:, s0:s0 + 2, c * BS:(c + 1) * BS],
                        start=False, stop=(c == NSTRIP - 1),
                        skip_group_check=True)
                for s in (0, 1):
                    tt = t + s
                    t1a = t1[0:10, s * BS:(s + 1) * BS]
                    t1b = t1[32:42, s * BS:(s + 1) * BS]
                    # correction channel: g1 = t1b * 2^-16 (ACT, PSUM-capable)
                    g1 = state.tile([10, BS], F32, tag="g1")
                    nc.scalar.mul(g1, t1b, float(2.0 ** (-S_FOLD)))
                    # h = beta*mem + t1a   (one PSUM operand)
                    h = state.tile([10, BS], F32, tag="h")
                    nc.vector.scalar_tensor_tensor(
                        out=h, in0=mem, scalar=BETA, in1=t1a,
                        op0=ALU.mult, op1=ALU.add)
                    u = state.tile([10, BS], F32, tag="u")
                    nc.vector.tensor_tensor(out=u, in0=h, in1=g1, op=ALU.add)
                    mem2 = state.tile([10, BS], F32, tag="mem")
                    nc.vector.scalar_tensor_tensor(
                        out=mem2, in0=mem, scalar=THR, in1=u,
                        op0=ALU.is_le, op1=ALU.mult)
                    sg = state.tile([10, BS], F32, tag="sg")
                    nc.scalar.activation(out=sg, in_=mem2, func=ACTF.Sign,
                                         bias=negthr, scale=1.0)
                    q = state.tile([10, BS], F32, tag="q")
                    nc.scalar.mul(q, sg, float(w_time[tt]) * 0.5)
                    nc.gpsimd.tensor_tensor(out=ft, in0=ft, in1=q, op=ALU.add)
                    mem = mem2

        # ---- output head ----
        d01 = ps_lg.tile([1, BS], F32)
        nc.tensor.matmul(d01, lhsT=wdiff, rhs=ft,
                         start=True, stop=True, skip_group_check=True)
        p0 = outp.tile([1, BS], F32, tag="p0")
        p1 = outp.tile([1, BS], F32, tag="p1")
        nc.scalar.activation(out=p0, in_=d01, func=ACTF.Sigmoid,
                             bias=csig_n, scale=-1.0)
        nc.scalar.activation(out=p1, in_=d01, func=ACTF.Sigmoid,
                             bias=csig_p, scale=1.0)
        nc.sync.dma_start(out=out_d[0:1, :], in_=p0)
        nc.sync.dma_start(out=out_d[1:2, :], in_=p1)
    nc.finalize()
    return nc


def _prep_weights(w_ant, b_ant, w_hid, b_hid, w_time, b_time, w_out, b_out):
    import ml_dtypes
    w_ant = np.asarray(w_ant, np.float64)
    w_hid = np.asarray(w_hid, np.float64)
    w_out = np.asarray(w_out, np.float64)
    b_out = np.asarray(b_out, np.float64)
    w_time64 = np.asarray(w_time, np.float64)
    wc = (w_ant[:, None, None] * w_hid.T[None, :, :]).reshape(AD, H)
    # strip-major [128, c, h]
    wst = wc.reshape(NSTRIP, 128, H).transpose(1, 0, 2)      # [128, 8, 10]
    w16 = wst.astype(np.float16).astype(np.float64)
    wres = ((wst - w16) * 2.0 ** S_FOLD).astype(np.float16)
    whi = np.zeros((128, 336), np.float16)
    for c in range(NSTRIP):
        whi[:, c * 42:c * 42 + 10] = w16[:, c, :].astype(np.float16)
        whi[:, c * 42 + 32:c * 42 + 42] = wres[:, c, :]
    wlo8 = (wst * 2.0 ** S_WLO).astype(
        ml_dtypes.float8_e4m3 if LO_FP8 else np.float16)
    wlo = np.zeros((128, 336), wlo8.dtype)
    for c in range(NSTRIP):
        wlo[:, c * 42 + 32:c * 42 + 42] = wlo8[:, c, :]

    bc = float(b_ant) * w_hid.sum(axis=1) + np.asarray(b_hid, np.float64)
    bc16 = bc.astype(np.float16).astype(np.float64)
    bcres = ((bc - bc16) * 2.0 ** S_FOLD).astype(np.float16)
    wext = np.zeros((1, 554), np.float16)
    wext[0, 0:512] = 1.0
    wext[0, 512:522] = bc16.astype(np.float16)
    wext[0, 544:554] = bcres

    wdiff = w_out[1] - w_out[0]
    c_sig = (float(b_time) + 0.5 * w_time64.sum()) * wdiff.sum() \
        + (b_out[1] - b_out[0])
    wf32 = np.zeros((10, 4), np.float32)
    wf32[0:10, 0] = wdiff.astype(np.float32)
    wf32[0, 1] = np.float32(c_sig)
    wf32[0, 2] = np.float32(-c_sig)
    wf32[0:10, 3] = np.float32(-THR)
    scalars = {"w_time": [float(v) for v in np.asarray(w_time, np.float32)]}
    return scalars, {"whi": whi, "wlo": wlo, "wext": wext, "wf32": wf32}


def _transpose_core(xc):
    """[256, 90, 4, 256] -> (hi fp16, lo) both [128, 90, 2048]."""
    import ml_dtypes
    v = np.ascontiguousarray(
        xc.reshape(BS, T, NSTRIP, 128).transpose(3, 1, 2, 0)
        .reshape(128, T, 2 * AD), dtype=np.float32)
    hi = v.astype(np.float16)
    lo = (v - hi.astype(np.float32)) * np.float32(2.0 ** S_LO)
    lo = lo.astype(ml_dtypes.float8_e4m3 if LO_FP8 else np.float16)
    return hi, lo


_CACHE = {}


def kernel(x, w_ant, b_ant, w_hid, b_hid, w_time, b_time, w_out, b_out):
    x = np.asarray(x, np.float32)
    assert x.shape == (B, T, A, D), x.shape
    scalars, consts = _prep_weights(w_ant, b_ant, w_hid, b_hid, w_time,
                                    b_time, w_out, b_out)
    key = (tuple(scalars["w_time"]), consts["wext"].tobytes(),
           consts["wf32"].tobytes(), consts["whi"].tobytes())
    nc = _CACHE.get(key)
    if nc is None:
        nc = _build(scalars)
        _CACHE[key] = nc
    in_maps = []
    for i in range(N_CORES):
        hi, lo = _transpose_core(x[i * BS:(i + 1) * BS])
        m = {"xhi": hi, "xlo": lo}
        m.update(consts)
        in_maps.append(m)
    r = run_bass_kernel_spmd(nc, in_maps, core_ids=list(range(N_CORES)))
    out = np.concatenate(
        [np.asarray(r.results[i]["out"]).T for i in range(N_CORES)], axis=0)
    return np.ascontiguousarray(out, dtype=np.float32)
